# revision 12
# baseline (speedup 1.0000x reference)
"""MultiHeadAttention Trainium2 kernel — 8-way (batch x head-half) sharded.

Sharding: core c <-> (batch b=c//2, head-half hh=c%2). Each core computes
QKV projections for its 512-dim head slice, attention for its 8 heads over
its batch, and a partial output projection; the host sums the two partials
per batch and adds bo.

Device layout ("T" = feature-on-partition transposed):
  xqT/xkT/xvT [1024,2048] bf16, maskT [2048,2048] bf16 (mask[b].T),
  wqT/wkT/wvT [1024,512] bf16, woT [512,1024] bf16, biases [1,512] bf16.
  Scores are computed transposed (scoresT[s,t]) so softmax-normalization
  uses ones-matmuls for the partition-dim row sums; exp runs on ACT with
  the 1/sqrt(dk) scale folded in; the 0/1 mask is applied multiplicatively
  after exp (exact: exp(-60000) == 0 in fp32).

Pipeline: per (half, pair) the two heads ping-pong through two [128,1024]
score-PSUM buffers so ACT (exp) stays saturated; K=64 score matmuls of the
two heads are row-packed, attn@v matmuls of the two heads are col-packed,
row-sum matmuls col-packed 4-wide. Projections for pairs 1..3 are emitted
inside earlier attention blocks to run in the PE shadow of exp.
"""

import numpy as np
import ml_dtypes

BF16 = ml_dtypes.bfloat16

B, T, DIM = 4, 2048, 1024
NUM_HEAD, D_K = 16, 64
N_CORES = 8
HPC = NUM_HEAD // 2          # heads per core = 8
DSL = HPC * D_K              # dim slice per core = 512
NPAIR = HPC // 2             # head pairs per core = 4
KT = DIM // 128              # k-tiles per full dim = 8
ST = T // 128                # s(key)-tiles = 16
NHALF = 2
HALF = T // NHALF            # 1024
NCH = HALF // 512            # 2

_PROG = None
_PROG_HAS_BIAS = None


def _emit(ctx, tc, d, with_bias):
    import concourse.bass as bass
    import concourse.mybir as mybir

    nc = tc.nc
    f32 = mybir.dt.float32
    bf16 = mybir.dt.bfloat16
    EXP = mybir.ActivationFunctionType.Exp

    const = ctx.enter_context(tc.tile_pool(name="const", bufs=1))
    res = ctx.enter_context(tc.tile_pool(name="res", bufs=1))
    xt_pool = ctx.enter_context(tc.tile_pool(name="xt", bufs=10))
    w_pool = ctx.enter_context(tc.tile_pool(name="w", bufs=10))
    mask_pool = ctx.enter_context(tc.tile_pool(name="mask", bufs=3))
    e_pool = ctx.enter_context(tc.tile_pool(name="e", bufs=2))
    u_pool = ctx.enter_context(tc.tile_pool(name="u", bufs=2))
    small = ctx.enter_context(tc.tile_pool(name="small", bufs=2))
    yt_pool = ctx.enter_context(tc.tile_pool(name="yt", bufs=2))
    ps_sc = ctx.enter_context(tc.tile_pool(name="ps_sc", bufs=2, space="PSUM"))
    ps_po = ctx.enter_context(tc.tile_pool(name="ps_po", bufs=1, space="PSUM"))
    ps_rs = ctx.enter_context(tc.tile_pool(name="ps_rs", bufs=1, space="PSUM"))
    ps_yt = ctx.enter_context(tc.tile_pool(name="ps_yt", bufs=1, space="PSUM"))

    ones = const.tile([128, 512 if with_bias else 128], bf16)
    nc.vector.memset(ones, 1.0)
    b_sb = {}
    if with_bias:
        for nm in ("bq", "bk", "bv"):
            t_ = const.tile([1, DSL], bf16, name=f"{nm}_sb")
            nc.sync.dma_start(out=t_, in_=d[nm])
            b_sb[nm] = t_
    wo_sb = []
    for p in range(NPAIR):
        t_ = res.tile([128, DIM], bf16, name=f"wo{p}")
        nc.sync.dma_start(out=t_, in_=d["woT"][p * 128:(p + 1) * 128, :])
        wo_sb.append(t_)

    # ---- input / weight loads ---------------------------------------------
    x_sb = {}
    w_sb = {}
    for which in ("xqT", "xkT", "xvT"):
        x_sb[which] = []
        for kk in range(KT):
            xtile = xt_pool.tile([128, T], bf16, tag=f"xt{which}",
                                 name=f"x{which}{kk}", bufs=KT)
            x_sb[which].append(xtile)
    for c in range(T // 512):
        for which in ("xqT", "xkT", "xvT"):
            for kk in range(KT):
                nc.sync.dma_start(
                    out=x_sb[which][kk][:, c * 512:(c + 1) * 512],
                    in_=d[which][kk * 128:(kk + 1) * 128, c * 512:(c + 1) * 512])
    for which in ("wvT",):
        w_sb[which] = []
        for kk in range(KT):
            wtile = w_pool.tile([128, DSL], bf16, tag=f"w{which}",
                                name=f"w{which}{kk}", bufs=KT)
            nc.sync.dma_start(out=wtile, in_=d[which][kk * 128:(kk + 1) * 128, :])
            w_sb[which].append(wtile)

    wchunks = {}

    def load_w_chunks(wname, p):
        key = (wname, p)
        if key in wchunks:
            return wchunks[key]
        tiles = []
        for kk in range(KT):
            wtile = w_pool.tile([128, 128], bf16, tag="wc",
                                name=f"wc{wname}{p}{kk}", bufs=2 * KT)
            nc.sync.dma_start(
                out=wtile,
                in_=d[wname][kk * 128:(kk + 1) * 128, p * 128:(p + 1) * 128])
            tiles.append(wtile)
        wchunks[key] = tiles
        return tiles

    qh_sb = [None] * NPAIR
    kh_sb = [None] * NPAIR
    vh_sb = [None] * ST

    def proj_qk_group(p, which, n):
        """One 512-chunk of the Q or K projection for pair p."""
        wname = "wqT" if which == "q" else "wkT"
        xname = "xqT" if which == "q" else "xkT"
        bname = "bq" if which == "q" else "bk"
        lst = qh_sb if which == "q" else kh_sb
        if lst[p] is None:
            lst[p] = res.tile([128, T], bf16, name=f"{which}h{p}")
        wc = load_w_chunks(wname, p)
        ps = ps_yt.tile([128, 512], f32, tag="yt", name="ps_proj")
        for kk in range(KT):
            nc.tensor.matmul(
                ps,
                wc[kk],
                x_sb[xname][kk][:, n * 512:(n + 1) * 512],
                start=(kk == 0), stop=(kk == KT - 1 and not with_bias),
            )
        if with_bias:
            nc.tensor.matmul(
                ps, b_sb[bname][0:1, p * 128:(p + 1) * 128],
                ones[0:1, 0:512], start=False, stop=True,
            )
        nc.vector.tensor_copy(lst[p][:, n * 512:(n + 1) * 512], ps)

    def proj_v_group(m):
        vtile = res.tile([128, DSL], bf16, name=f"vh{m}")
        ps = ps_yt.tile([128, 512], f32, tag="yt", name="ps_projv")
        for kk in range(KT):
            nc.tensor.matmul(
                ps, x_sb["xvT"][kk][:, m * 128:(m + 1) * 128], w_sb["wvT"][kk],
                start=(kk == 0), stop=(kk == KT - 1 and not with_bias),
            )
        if with_bias:
            nc.tensor.matmul(ps, ones[0:1, 0:128], b_sb["bv"][0:1, :],
                             start=False, stop=True)
        nc.vector.tensor_copy(vtile, ps)
        vh_sb[m] = vtile

    # Work queue of deferred PE work to stuff into attention ACT shadows.
    shadow_work = []
    for p in range(1, NPAIR):
        for which in ("q", "k"):
            for n in range(T // 512):
                shadow_work.append(
                    lambda p=p, w=which, n=n: proj_qk_group(p, w, n))

    # Upfront: pair-0 Q/K (chunk-granular so attention can start early), V.
    for n in (0, 1):
        proj_qk_group(0, "q", n)
    for n in (0, 1):
        proj_qk_group(0, "k", n)
    for n in (2, 3):
        proj_qk_group(0, "q", n)
    for n in (2, 3):
        proj_qk_group(0, "k", n)
    for m in range(ST):
        proj_v_group(m)

    # ---- attention ---------------------------------------------------------
    def attention_block(hf, p, budget, pending):
        """One (half, pair): 16 s-tiles, two ping-ponged head chains."""
        qh, kh, vcol = qh_sb[p], kh_sb[p], p * 128
        po = ps_po.tile([128, HALF], f32, tag="po", name="ps_out")
        rs = ps_rs.tile([128, 512], f32, tag="rs", name="ps_rsum")
        U_prev = [None, None]

        def head_pe(hi, st):
            # scores(st) then attn@v/rowsums(st-1) for head hi
            hp = slice(hi * 64, (hi + 1) * 64)
            sc = ps_sc.tile([128, HALF], f32, tag="sc", name="ps_sc")
            for n in range(NCH):
                nc.tensor.matmul(
                    sc[:, n * 512:(n + 1) * 512],
                    kh[hp, st * 128:(st + 1) * 128],
                    qh[hp, hf * HALF + n * 512: hf * HALF + (n + 1) * 512],
                    start=True, stop=True,
                )
            if U_prev[hi] is not None:
                for n in range(NCH):
                    nc.tensor.matmul(
                        po[hi * 64: hi * 64 + 64, n * 512:(n + 1) * 512],
                        vh_sb[st - 1][:, vcol + hi * 64: vcol + (hi + 1) * 64],
                        U_prev[hi][:, n * 512:(n + 1) * 512],
                        start=(st == 1), stop=False,
                        tile_position=(0, hi * 64),
                    )
                for n in range(NCH):
                    r = hi * 64 + n * 32
                    nc.tensor.matmul(
                        rs[r: r + 1, 0:512],
                        ones[:, 0:1],
                        U_prev[hi][:, n * 512:(n + 1) * 512],
                        start=(st == 1), stop=False,
                        tile_position=(0, r),
                    )
            return sc

        def head_act(hi, sc):
            E = e_pool.tile([128, HALF], bf16, tag="e", name="E")
            nc.scalar.activation(E, sc, EXP, scale=0.125)
            U = u_pool.tile([128, HALF], bf16, tag="u", name="U")
            nc.vector.tensor_mul(U, E, msk_cur[0])
            U_prev[hi] = U

        msk_cur = [None]
        for st in range(ST):
            msk = mask_pool.tile([128, HALF], bf16, tag="mask", name="msk")
            nc.sync.dma_start(
                out=msk,
                in_=d["maskT"][st * 128:(st + 1) * 128,
                               hf * HALF:(hf + 1) * HALF],
            )
            msk_cur[0] = msk
            scA = head_pe(0, st)
            if st == 1 and pending:
                pending.pop(0)()
            scB = head_pe(1, st)
            if budget and st % 2 == 0 and shadow_work:
                shadow_work.pop(0)()
            head_act(0, scA)
            head_act(1, scB)
        # final tile attn@v + row sums
        for hi in range(2):
            for n in range(NCH):
                nc.tensor.matmul(
                    po[hi * 64: hi * 64 + 64, n * 512:(n + 1) * 512],
                    vh_sb[ST - 1][:, vcol + hi * 64: vcol + (hi + 1) * 64],
                    U_prev[hi][:, n * 512:(n + 1) * 512],
                    start=False, stop=True,
                    tile_position=(0, hi * 64),
                )
            for n in range(NCH):
                r = hi * 64 + n * 32
                nc.tensor.matmul(
                    rs[r: r + 1, 0:512],
                    ones[:, 0:1],
                    U_prev[hi][:, n * 512:(n + 1) * 512],
                    start=False, stop=True,
                    tile_position=(0, r),
                )
        # head-chain tail: recip + po evac now (DVE); bc + normalize deferred
        rc = small.tile([128, 512], bf16, tag="rc", name="rc")
        with nc.allow_low_precision(reason="softmax normalizer in bf16"):
            nc.vector.reciprocal(rc, rs)
        po_sb = small.tile([128, HALF], bf16, tag="po_sb", name="po_sb")
        with nc.allow_low_precision(reason="attn out evac in bf16"):
            nc.vector.tensor_copy(po_sb, po)
        on_t = xt_pool.tile([128, HALF], bf16, tag="xtxvT",
                            name=f"on{hf}_{p}", bufs=KT)

        def finisher():
            bc = ps_sc.tile([128, HALF], f32, tag="sc", name="ps_bc")
            for hi2 in range(2):
                for n in range(NCH):
                    r = hi2 * 64 + n * 32
                    nc.tensor.matmul(
                        bc[hi2 * 64: hi2 * 64 + 64, n * 512:(n + 1) * 512],
                        ones[r: r + 1, 0:64],
                        rc[r: r + 1, :],
                        start=True, stop=True,
                        tile_position=(r, hi2 * 64),
                    )
            nc.vector.tensor_mul(on_t, po_sb, bc)

        return on_t, finisher

    pending = []
    for hf in range(NHALF):
        on_sb = []
        for p in range(NPAIR):
            on_t, fin = attention_block(hf, p, budget=True, pending=pending)
            on_sb.append(on_t)
            pending.append(fin)
        while pending:
            pending.pop(0)()
        # output projection for this half
        last = hf == NHALF - 1
        gi = 0
        for m in range(DIM // 128):
            for n in range(NCH):
                if last and gi % 2 == 1:
                    ps = ps_rs.tile([128, 512], f32, tag="rs", name="ps_yt2")
                else:
                    ps = ps_yt.tile([128, 512], f32, tag="yt", name="ps_yt")
                gi += 1
                for p in range(NPAIR):
                    nc.tensor.matmul(
                        ps,
                        wo_sb[p][:, m * 128:(m + 1) * 128],
                        on_sb[p][:, n * 512:(n + 1) * 512],
                        start=(p == 0), stop=(p == NPAIR - 1),
                    )
                yt_t = yt_pool.tile([128, 512], f32, tag="yt_sb", name="yt_sb")
                nc.vector.tensor_copy(yt_t, ps)
                nc.sync.dma_start(
                    out=d["yT"][m * 128:(m + 1) * 128,
                                hf * HALF + n * 512: hf * HALF + (n + 1) * 512],
                    in_=yt_t,
                )


def _build(with_bias=False):
    global _PROG, _PROG_HAS_BIAS
    if _PROG is not None and _PROG_HAS_BIAS == with_bias:
        return _PROG
    import concourse.mybir as mybir
    from concourse import bacc
    from concourse.tile import TileContext
    from contextlib import ExitStack

    nc = bacc.Bacc("TRN2", target_bir_lowering=False, debug=False)
    f32 = mybir.dt.float32
    bf16 = mybir.dt.bfloat16
    d = {
        "xqT": nc.dram_tensor("xqT", [DIM, T], bf16, kind="ExternalInput").ap(),
        "xkT": nc.dram_tensor("xkT", [DIM, T], bf16, kind="ExternalInput").ap(),
        "xvT": nc.dram_tensor("xvT", [DIM, T], bf16, kind="ExternalInput").ap(),
        "maskT": nc.dram_tensor("maskT", [T, T], bf16, kind="ExternalInput").ap(),
        "wqT": nc.dram_tensor("wqT", [DIM, DSL], bf16, kind="ExternalInput").ap(),
        "wkT": nc.dram_tensor("wkT", [DIM, DSL], bf16, kind="ExternalInput").ap(),
        "wvT": nc.dram_tensor("wvT", [DIM, DSL], bf16, kind="ExternalInput").ap(),
        "woT": nc.dram_tensor("woT", [DSL, DIM], bf16, kind="ExternalInput").ap(),
        "yT": nc.dram_tensor("yT", [DIM, T], f32, kind="ExternalOutput").ap(),
    }
    if with_bias:
        for nm in ("bq", "bk", "bv"):
            d[nm] = nc.dram_tensor(nm, [1, DSL], bf16, kind="ExternalInput").ap()
    with TileContext(nc) as tc:
        with ExitStack() as es:
            _emit(es, tc, d, with_bias)
    nc.compile()
    _PROG = nc
    _PROG_HAS_BIAS = with_bias
    return nc


def prepare_in_maps(q, k, v, mask, Wq, bq, Wk, bk, Wv, bv, Wo, bo):
    q, k, v = (np.asarray(x, np.float32) for x in (q, k, v))
    mask = np.asarray(mask)
    with_bias = bool(
        np.any(np.asarray(bq)) or np.any(np.asarray(bk)) or np.any(np.asarray(bv))
    )
    in_maps = []
    for c in range(N_CORES):
        b, hh = c // 2, c % 2
        rows = slice(hh * DSL, (hh + 1) * DSL)
        m = {
            "xqT": np.ascontiguousarray(q[b].T).astype(BF16),
            "xkT": np.ascontiguousarray(k[b].T).astype(BF16),
            "xvT": np.ascontiguousarray(v[b].T).astype(BF16),
            "maskT": np.ascontiguousarray(mask[b].T).astype(BF16),
            "wqT": np.ascontiguousarray(np.asarray(Wq, np.float32)[rows, :].T).astype(BF16),
            "wkT": np.ascontiguousarray(np.asarray(Wk, np.float32)[rows, :].T).astype(BF16),
            "wvT": np.ascontiguousarray(np.asarray(Wv, np.float32)[rows, :].T).astype(BF16),
            "woT": np.ascontiguousarray(np.asarray(Wo, np.float32)[:, rows].T).astype(BF16),
        }
        if with_bias:
            m["bq"] = np.asarray(bq, np.float32)[rows].reshape(1, DSL).astype(BF16)
            m["bk"] = np.asarray(bk, np.float32)[rows].reshape(1, DSL).astype(BF16)
            m["bv"] = np.asarray(bv, np.float32)[rows].reshape(1, DSL).astype(BF16)
        in_maps.append(m)
    return in_maps, with_bias


def run_spmd(in_maps, with_bias=False, trace=False, trace_cores=None):
    from concourse.bass_utils import run_bass_kernel_spmd

    nc = _build(with_bias)
    return run_bass_kernel_spmd(
        nc, in_maps, core_ids=list(range(N_CORES)), trace=trace,
        trace_cores=trace_cores,
    )


def assemble(results, bo):
    out = np.empty((B, T, DIM), np.float32)
    for b in range(B):
        yt = results[2 * b]["yT"].astype(np.float32) + results[2 * b + 1]["yT"].astype(np.float32)
        out[b] = yt.T
    out += np.asarray(bo, np.float32)[None, None, :]
    return out


def kernel(q, k, v, mask, Wq, bq, Wk, bk, Wv, bv, Wo, bo):
    in_maps, with_bias = prepare_in_maps(q, k, v, mask, Wq, bq, Wk, bk, Wv, bv, Wo, bo)
    res = run_spmd(in_maps, with_bias=with_bias, trace=False)
    return assemble(res.results, bo)


# revision 13
# speedup vs baseline: 1.1968x; 1.1968x over previous
"""MultiHeadAttention Trainium2 kernel — 8-way (batch x head-half) sharded.

Sharding: core c <-> (batch b=c//2, head-half hh=c%2). Each core computes
QKV projections for its 512-dim head slice, attention for its 8 heads over
its batch, and a partial output projection; the host sums the two partials
per batch and adds bo.

Device layout ("T" = feature-on-partition transposed):
  xqT/xkT/xvT [1024,2048] bf16, maskT [2048,2048] bf16 (mask[b].T),
  wqT/wkT/wvT [1024,512] bf16, woT [512,1024] bf16, biases [1,512] bf16.
  Scores are computed transposed (scoresT[s,t]) so softmax-normalization
  uses ones-matmuls for the partition-dim row sums; exp runs on ACT with
  the 1/sqrt(dk) scale folded in; the 0/1 mask is applied multiplicatively
  after exp (exact: exp(-60000) == 0 in fp32).

Pipeline: per (half, pair) the two heads ping-pong through two [128,1024]
score-PSUM buffers so ACT (exp) stays saturated; K=64 score matmuls of the
two heads are row-packed, attn@v matmuls of the two heads are col-packed,
row-sum matmuls col-packed 4-wide. Projections for pairs 1..3 are emitted
inside earlier attention blocks to run in the PE shadow of exp.
"""

import numpy as np
import ml_dtypes

BF16 = ml_dtypes.bfloat16

B, T, DIM = 4, 2048, 1024
NUM_HEAD, D_K = 16, 64
N_CORES = 8
HPC = NUM_HEAD // 2          # heads per core = 8
DSL = HPC * D_K              # dim slice per core = 512
NPAIR = HPC // 2             # head pairs per core = 4
KT = DIM // 128              # k-tiles per full dim = 8
ST = T // 128                # s(key)-tiles = 16
NHALF = 2
HALF = T // NHALF            # 1024
NCH = HALF // 512            # 2

_PROG = None
_PROG_HAS_BIAS = None


def _emit(ctx, tc, d, with_bias):
    import concourse.bass as bass
    import concourse.mybir as mybir

    nc = tc.nc
    f32 = mybir.dt.float32
    bf16 = mybir.dt.bfloat16
    EXP = mybir.ActivationFunctionType.Exp

    const = ctx.enter_context(tc.tile_pool(name="const", bufs=1))
    res = ctx.enter_context(tc.tile_pool(name="res", bufs=1))
    xt_pool = ctx.enter_context(tc.tile_pool(name="xt", bufs=10))
    w_pool = ctx.enter_context(tc.tile_pool(name="w", bufs=10))
    mask_pool = ctx.enter_context(tc.tile_pool(name="mask", bufs=3))
    e_pool = ctx.enter_context(tc.tile_pool(name="e", bufs=2))
    u_pool = ctx.enter_context(tc.tile_pool(name="u", bufs=2))
    small = ctx.enter_context(tc.tile_pool(name="small", bufs=2))
    yt_pool = ctx.enter_context(tc.tile_pool(name="yt", bufs=2))
    ps_sc = ctx.enter_context(tc.tile_pool(name="ps_sc", bufs=2, space="PSUM"))
    ps_po = ctx.enter_context(tc.tile_pool(name="ps_po", bufs=1, space="PSUM"))
    ps_rs = ctx.enter_context(tc.tile_pool(name="ps_rs", bufs=1, space="PSUM"))
    ps_yt = ctx.enter_context(tc.tile_pool(name="ps_yt", bufs=1, space="PSUM"))

    ones = const.tile([128, 512 if with_bias else 128], bf16)
    nc.vector.memset(ones, 1.0)
    b_sb = {}
    if with_bias:
        for nm in ("bq", "bk", "bv"):
            t_ = const.tile([1, DSL], bf16, name=f"{nm}_sb")
            nc.sync.dma_start(out=t_, in_=d[nm])
            b_sb[nm] = t_
    wo_sb = []
    for p in range(NPAIR):
        t_ = res.tile([128, DIM], bf16, name=f"wo{p}")
        nc.sync.dma_start(out=t_, in_=d["woT"][p * 128:(p + 1) * 128, :])
        wo_sb.append(t_)

    # ---- input / weight loads ---------------------------------------------
    x_sb = {}
    w_sb = {}
    for which in ("xqT", "xkT", "xvT"):
        x_sb[which] = []
        for kk in range(KT):
            xtile = xt_pool.tile([128, T], bf16, tag=f"xt{which}",
                                 name=f"x{which}{kk}", bufs=KT)
            x_sb[which].append(xtile)
    for which in ("xqT", "xkT", "xvT"):
        for kk in range(KT):
            nc.sync.dma_start(out=x_sb[which][kk],
                              in_=d[which][kk * 128:(kk + 1) * 128, :])
    for which in ("wvT",):
        w_sb[which] = []
        for kk in range(KT):
            wtile = w_pool.tile([128, DSL], bf16, tag=f"w{which}",
                                name=f"w{which}{kk}", bufs=KT)
            nc.sync.dma_start(out=wtile, in_=d[which][kk * 128:(kk + 1) * 128, :])
            w_sb[which].append(wtile)

    wchunks = {}

    def load_w_chunks(wname, p):
        key = (wname, p)
        if key in wchunks:
            return wchunks[key]
        tiles = []
        for kk in range(KT):
            wtile = w_pool.tile([128, 128], bf16, tag="wc",
                                name=f"wc{wname}{p}{kk}", bufs=2 * KT)
            nc.sync.dma_start(
                out=wtile,
                in_=d[wname][kk * 128:(kk + 1) * 128, p * 128:(p + 1) * 128])
            tiles.append(wtile)
        wchunks[key] = tiles
        return tiles

    qh_sb = [None] * NPAIR
    kh_sb = [None] * NPAIR
    vh_sb = [None] * ST

    def proj_qk_group(p, which, n):
        """One 512-chunk of the Q or K projection for pair p."""
        wname = "wqT" if which == "q" else "wkT"
        xname = "xqT" if which == "q" else "xkT"
        bname = "bq" if which == "q" else "bk"
        lst = qh_sb if which == "q" else kh_sb
        if lst[p] is None:
            lst[p] = res.tile([128, T], bf16, name=f"{which}h{p}")
        wc = load_w_chunks(wname, p)
        ps = ps_yt.tile([128, 512], f32, tag="yt", name="ps_proj")
        for kk in range(KT):
            nc.tensor.matmul(
                ps,
                wc[kk],
                x_sb[xname][kk][:, n * 512:(n + 1) * 512],
                start=(kk == 0), stop=(kk == KT - 1 and not with_bias),
            )
        if with_bias:
            nc.tensor.matmul(
                ps, b_sb[bname][0:1, p * 128:(p + 1) * 128],
                ones[0:1, 0:512], start=False, stop=True,
            )
        nc.vector.tensor_copy(lst[p][:, n * 512:(n + 1) * 512], ps)

    def proj_v_group(m):
        vtile = res.tile([128, DSL], bf16, name=f"vh{m}")
        ps = ps_yt.tile([128, 512], f32, tag="yt", name="ps_projv")
        for kk in range(KT):
            nc.tensor.matmul(
                ps, x_sb["xvT"][kk][:, m * 128:(m + 1) * 128], w_sb["wvT"][kk],
                start=(kk == 0), stop=(kk == KT - 1 and not with_bias),
            )
        if with_bias:
            nc.tensor.matmul(ps, ones[0:1, 0:128], b_sb["bv"][0:1, :],
                             start=False, stop=True)
        nc.vector.tensor_copy(vtile, ps)
        vh_sb[m] = vtile

    # Work queue of deferred PE work to stuff into attention ACT shadows.
    shadow_work = []
    for p in range(1, NPAIR):
        for which in ("q", "k"):
            for n in range(T // 512):
                shadow_work.append(
                    lambda p=p, w=which, n=n: proj_qk_group(p, w, n))

    # Upfront: pair-0 Q/K (chunk-granular so attention can start early), V.
    for n in (0, 1):
        proj_qk_group(0, "q", n)
    for n in (0, 1):
        proj_qk_group(0, "k", n)
    for n in (2, 3):
        proj_qk_group(0, "q", n)
    for n in (2, 3):
        proj_qk_group(0, "k", n)
    for m in range(ST):
        proj_v_group(m)

    # ---- attention ---------------------------------------------------------
    def attention_block(hf, p, budget, pending):
        """One (half, pair): 16 s-tiles, two ping-ponged head chains."""
        qh, kh, vcol = qh_sb[p], kh_sb[p], p * 128
        po = ps_po.tile([128, HALF], f32, tag="po", name="ps_out")
        rs = ps_rs.tile([128, 512], f32, tag="rs", name="ps_rsum")
        U_prev = [None, None]

        def tile_pe(st):
            # scores(st) for both heads (64-row mode, alternating T0/T8),
            # then attn@v(st-1) both heads (128-mode, alternating col), then
            # rowsums(st-1) col-packed 4-wide: 3 mode groups per s-tile.
            scs = []
            for hi in range(2):
                scs.append(ps_sc.tile([128, HALF], f32, tag="sc",
                                      name="ps_sc"))
            for n in range(NCH):
                for hi in range(2):
                    hp = slice(hi * 64, (hi + 1) * 64)
                    nc.tensor.matmul(
                        scs[hi][:, n * 512:(n + 1) * 512],
                        kh[hp, st * 128:(st + 1) * 128],
                        qh[hp, hf * HALF + n * 512: hf * HALF + (n + 1) * 512],
                        start=True, stop=True,
                    )
            if U_prev[0] is not None:
                for n in range(NCH):
                    for hi in range(2):
                        nc.tensor.matmul(
                            po[hi * 64: hi * 64 + 64, n * 512:(n + 1) * 512],
                            vh_sb[st - 1][:, vcol + hi * 64: vcol + (hi + 1) * 64],
                            U_prev[hi][:, n * 512:(n + 1) * 512],
                            start=(st == 1), stop=False,
                            tile_position=(0, hi * 64),
                        )
                for hi in range(2):
                    for n in range(NCH):
                        r = hi * 64 + n * 32
                        nc.tensor.matmul(
                            rs[r: r + 1, 0:512],
                            ones[:, 0:1],
                            U_prev[hi][:, n * 512:(n + 1) * 512],
                            start=(st == 1), stop=False,
                            tile_position=(0, r),
                        )
            return scs

        def head_act(hi, sc):
            E = e_pool.tile([128, HALF], bf16, tag="e", name="E")
            nc.scalar.activation(E, sc, EXP, scale=0.125)
            U = u_pool.tile([128, HALF], bf16, tag="u", name="U")
            nc.vector.tensor_mul(U, E, msk_cur[0])
            U_prev[hi] = U

        msk_cur = [None]
        for st in range(ST):
            msk = mask_pool.tile([128, HALF], bf16, tag="mask", name="msk")
            nc.sync.dma_start(
                out=msk,
                in_=d["maskT"][st * 128:(st + 1) * 128,
                               hf * HALF:(hf + 1) * HALF],
            )
            msk_cur[0] = msk
            scA, scB = tile_pe(st)
            if budget and st % 2 == 0 and shadow_work:
                shadow_work.pop(0)()
            head_act(0, scA)
            head_act(1, scB)
        # final tile attn@v + row sums
        for n in range(NCH):
            for hi in range(2):
                nc.tensor.matmul(
                    po[hi * 64: hi * 64 + 64, n * 512:(n + 1) * 512],
                    vh_sb[ST - 1][:, vcol + hi * 64: vcol + (hi + 1) * 64],
                    U_prev[hi][:, n * 512:(n + 1) * 512],
                    start=False, stop=True,
                    tile_position=(0, hi * 64),
                )
        for hi in range(2):
            for n in range(NCH):
                r = hi * 64 + n * 32
                nc.tensor.matmul(
                    rs[r: r + 1, 0:512],
                    ones[:, 0:1],
                    U_prev[hi][:, n * 512:(n + 1) * 512],
                    start=False, stop=True,
                    tile_position=(0, r),
                )
        # head-chain tail: recip + po evac now (DVE); bc + normalize deferred
        rc = small.tile([128, 512], bf16, tag="rc", name="rc")
        with nc.allow_low_precision(reason="softmax normalizer in bf16"):
            nc.vector.reciprocal(rc, rs)
        po_sb = small.tile([128, HALF], bf16, tag="po_sb", name="po_sb")
        with nc.allow_low_precision(reason="attn out evac in bf16"):
            nc.vector.tensor_copy(po_sb, po)
        on_t = xt_pool.tile([128, HALF], bf16, tag="xtxvT",
                            name=f"on{hf}_{p}", bufs=KT)

        def finisher():
            bc = ps_sc.tile([128, HALF], f32, tag="sc", name="ps_bc")
            for hi2 in range(2):
                for n in range(NCH):
                    r = hi2 * 64 + n * 32
                    nc.tensor.matmul(
                        bc[hi2 * 64: hi2 * 64 + 64, n * 512:(n + 1) * 512],
                        ones[r: r + 1, 0:64],
                        rc[r: r + 1, :],
                        start=True, stop=True,
                        tile_position=(r, hi2 * 64),
                    )
            nc.vector.tensor_mul(on_t, po_sb, bc)

        return on_t, finisher

    pending = []
    for hf in range(NHALF):
        on_sb = []
        for p in range(NPAIR):
            on_t, fin = attention_block(hf, p, budget=True, pending=pending)
            on_sb.append(on_t)
            if pending:
                pending.pop(0)()
            pending.append(fin)
        while pending:
            pending.pop(0)()
        # output projection for this half
        last = hf == NHALF - 1
        gi = 0
        for m in range(DIM // 128):
            for n in range(NCH):
                if last and gi % 2 == 1:
                    ps = ps_rs.tile([128, 512], f32, tag="rs", name="ps_yt2")
                else:
                    ps = ps_yt.tile([128, 512], f32, tag="yt", name="ps_yt")
                gi += 1
                for p in range(NPAIR):
                    nc.tensor.matmul(
                        ps,
                        wo_sb[p][:, m * 128:(m + 1) * 128],
                        on_sb[p][:, n * 512:(n + 1) * 512],
                        start=(p == 0), stop=(p == NPAIR - 1),
                    )
                yt_t = yt_pool.tile([128, 512], f32, tag="yt_sb", name="yt_sb")
                nc.vector.tensor_copy(yt_t, ps)
                nc.sync.dma_start(
                    out=d["yT"][m * 128:(m + 1) * 128,
                                hf * HALF + n * 512: hf * HALF + (n + 1) * 512],
                    in_=yt_t,
                )


def _build(with_bias=False):
    global _PROG, _PROG_HAS_BIAS
    if _PROG is not None and _PROG_HAS_BIAS == with_bias:
        return _PROG
    import concourse.mybir as mybir
    from concourse import bacc
    from concourse.tile import TileContext
    from contextlib import ExitStack

    nc = bacc.Bacc("TRN2", target_bir_lowering=False, debug=False)
    f32 = mybir.dt.float32
    bf16 = mybir.dt.bfloat16
    d = {
        "xqT": nc.dram_tensor("xqT", [DIM, T], bf16, kind="ExternalInput").ap(),
        "xkT": nc.dram_tensor("xkT", [DIM, T], bf16, kind="ExternalInput").ap(),
        "xvT": nc.dram_tensor("xvT", [DIM, T], bf16, kind="ExternalInput").ap(),
        "maskT": nc.dram_tensor("maskT", [T, T], bf16, kind="ExternalInput").ap(),
        "wqT": nc.dram_tensor("wqT", [DIM, DSL], bf16, kind="ExternalInput").ap(),
        "wkT": nc.dram_tensor("wkT", [DIM, DSL], bf16, kind="ExternalInput").ap(),
        "wvT": nc.dram_tensor("wvT", [DIM, DSL], bf16, kind="ExternalInput").ap(),
        "woT": nc.dram_tensor("woT", [DSL, DIM], bf16, kind="ExternalInput").ap(),
        "yT": nc.dram_tensor("yT", [DIM, T], f32, kind="ExternalOutput").ap(),
    }
    if with_bias:
        for nm in ("bq", "bk", "bv"):
            d[nm] = nc.dram_tensor(nm, [1, DSL], bf16, kind="ExternalInput").ap()
    with TileContext(nc) as tc:
        with ExitStack() as es:
            _emit(es, tc, d, with_bias)
    nc.compile()
    _PROG = nc
    _PROG_HAS_BIAS = with_bias
    return nc


def prepare_in_maps(q, k, v, mask, Wq, bq, Wk, bk, Wv, bv, Wo, bo):
    q, k, v = (np.asarray(x, np.float32) for x in (q, k, v))
    mask = np.asarray(mask)
    with_bias = bool(
        np.any(np.asarray(bq)) or np.any(np.asarray(bk)) or np.any(np.asarray(bv))
    )
    in_maps = []
    for c in range(N_CORES):
        b, hh = c // 2, c % 2
        rows = slice(hh * DSL, (hh + 1) * DSL)
        m = {
            "xqT": np.ascontiguousarray(q[b].T).astype(BF16),
            "xkT": np.ascontiguousarray(k[b].T).astype(BF16),
            "xvT": np.ascontiguousarray(v[b].T).astype(BF16),
            "maskT": np.ascontiguousarray(mask[b].T).astype(BF16),
            "wqT": np.ascontiguousarray(np.asarray(Wq, np.float32)[rows, :].T).astype(BF16),
            "wkT": np.ascontiguousarray(np.asarray(Wk, np.float32)[rows, :].T).astype(BF16),
            "wvT": np.ascontiguousarray(np.asarray(Wv, np.float32)[rows, :].T).astype(BF16),
            "woT": np.ascontiguousarray(np.asarray(Wo, np.float32)[:, rows].T).astype(BF16),
        }
        if with_bias:
            m["bq"] = np.asarray(bq, np.float32)[rows].reshape(1, DSL).astype(BF16)
            m["bk"] = np.asarray(bk, np.float32)[rows].reshape(1, DSL).astype(BF16)
            m["bv"] = np.asarray(bv, np.float32)[rows].reshape(1, DSL).astype(BF16)
        in_maps.append(m)
    return in_maps, with_bias


def run_spmd(in_maps, with_bias=False, trace=False, trace_cores=None):
    from concourse.bass_utils import run_bass_kernel_spmd

    nc = _build(with_bias)
    return run_bass_kernel_spmd(
        nc, in_maps, core_ids=list(range(N_CORES)), trace=trace,
        trace_cores=trace_cores,
    )


def assemble(results, bo):
    out = np.empty((B, T, DIM), np.float32)
    for b in range(B):
        yt = results[2 * b]["yT"].astype(np.float32) + results[2 * b + 1]["yT"].astype(np.float32)
        out[b] = yt.T
    out += np.asarray(bo, np.float32)[None, None, :]
    return out


def kernel(q, k, v, mask, Wq, bq, Wk, bk, Wv, bv, Wo, bo):
    in_maps, with_bias = prepare_in_maps(q, k, v, mask, Wq, bq, Wk, bk, Wv, bv, Wo, bo)
    res = run_spmd(in_maps, with_bias=with_bias, trace=False)
    return assemble(res.results, bo)
